# revision 1
# baseline (speedup 1.0000x reference)
"""GCN 3-layer kernel for Trainium2, 8-core SPMD.

Math (per layer, PyG GCN convention with self-loops, factorized):
    deg[d]  = indegree(d) + 1;  dinv = deg^-1/2
    y       = dinv[:,None] * (h @ W)                    (per-node scale)
    agg[d]  = sum_{e: dst[e]=d} y[src[e]]  + y[d]       (self-loop as edge)
    h_next  = dinv[:,None] * agg + b                    (+ relu on last layer)

Distribution: destination-sharded across 8 cores (6272 = 49*128 node slots
per core, padded to 50176 total).  Each core computes y for its own nodes,
an AllGather replicates the full y table (bf16) to every core's DRAM, then
each core gathers message rows with dma_gather and scatter-adds them with
one-hot matmuls on the PE (PSUM accumulation per 128-dst window).

dma_gather indices are int16, so the y table is addressed via two base
offsets (row 0 for src < 25088, row 17408 for src >= 25088; 50176-17408 =
32768 rows exactly covers the int16 range).
"""

import numpy as np
import ml_dtypes

N_NODES = 50000
N_CORES = 8
PER_CORE = 6272            # 49 * 128
N_PAD = PER_CORE * N_CORES # 50176
N_WIN = PER_CORE // 128    # 49
HI_BASE = 17408            # hi gather base row; 50176-17408 = 32768
LO_HI_SPLIT = 25088        # src < split -> lo stream, else hi
F = 128                    # feature width (layer3 padded 64->128)
F_OUT = 64
GROUP_WINDOWS = 5          # windows per gather chunk

BF16 = ml_dtypes.bfloat16


def _wrap_idx16(idx: np.ndarray) -> np.ndarray:
    """Wrap a flat int16 index stream into the [128, n/16] layout dma_gather
    expects (element i at [i%16, i//16], replicated across the 8 groups of
    16 partitions)."""
    n = len(idx)
    assert n % 128 == 0
    cols = n // 16
    out = np.empty((128, cols), np.int16)
    w = idx.reshape(cols, 16).T  # [16, cols]
    for g in range(8):
        out[g * 16:(g + 1) * 16, :] = w
    return out


def _preprocess(edge_index: np.ndarray):
    """Host-side graph prep: degree norm, dst-sharding, per-window edge
    streams (lo/hi by source row), block padding shared across cores."""
    src = edge_index[0].astype(np.int64)
    dst = edge_index[1].astype(np.int64)
    deg = np.bincount(dst, minlength=N_NODES).astype(np.float64) + 1.0
    dinv = (1.0 / np.sqrt(deg)).astype(np.float32)
    dinv_pad = np.ones(N_PAD, np.float32)
    dinv_pad[:N_NODES] = dinv

    # append self-edges
    selfn = np.arange(N_NODES, dtype=np.int64)
    src_a = np.concatenate([src, selfn])
    dst_a = np.concatenate([dst, selfn])

    core_of = dst_a // PER_CORE
    win_of = (dst_a % PER_CORE) // 128
    dloc_of = dst_a % 128
    is_lo = src_a < LO_HI_SPLIT

    # bucket edges: per (core, window, stream) lists of (idx16, dst_local)
    # sort once by (core, window)
    order = np.lexsort((dst_a, win_of, core_of))
    src_s, core_s, win_s, dloc_s, lo_s = (
        src_a[order], core_of[order], win_of[order], dloc_of[order], is_lo[order])

    # per (core, window, stream) counts
    counts = np.zeros((N_CORES, N_WIN, 2), np.int64)
    np.add.at(counts, (core_s, win_s, (~lo_s).astype(np.int64)), 1)
    # shared block counts per window (max over cores), at least 1 lo block
    blk_lo = np.maximum(1, -(-counts[:, :, 0].max(axis=0) // 128))  # [N_WIN]
    blk_hi = np.maximum(0, -(-counts[:, :, 1].max(axis=0) // 128))  # [N_WIN]

    # slot offsets within each stream
    off_lo = np.concatenate([[0], np.cumsum(blk_lo * 128)])
    off_hi = np.concatenate([[0], np.cumsum(blk_hi * 128)])
    n_lo, n_hi = int(off_lo[-1]), int(off_hi[-1])

    # fill per-core padded streams
    idx_lo = np.zeros((N_CORES, n_lo), np.int16)
    idx_hi = np.zeros((N_CORES, n_hi), np.int16)
    dl_lo = np.full((N_CORES, n_lo), 999.0, np.float32)
    dl_hi = np.full((N_CORES, n_hi), 999.0, np.float32)

    # boundaries of (core, window) groups in the sorted arrays
    keys = core_s * N_WIN + win_s
    bounds = np.searchsorted(keys, np.arange(N_CORES * N_WIN + 1))
    for c in range(N_CORES):
        for w in range(N_WIN):
            k = c * N_WIN + w
            sl = slice(bounds[k], bounds[k + 1])
            s_src = src_s[sl]; s_dl = dloc_s[sl]; s_lo = lo_s[sl]
            lo_src = s_src[s_lo]; lo_dl = s_dl[s_lo]
            hi_src = s_src[~s_lo]; hi_dl = s_dl[~s_lo]
            o = off_lo[w]
            idx_lo[c, o:o + len(lo_src)] = lo_src.astype(np.int16)
            dl_lo[c, o:o + len(lo_src)] = lo_dl
            o = off_hi[w]
            idx_hi[c, o:o + len(hi_src)] = (hi_src - HI_BASE).astype(np.int16)
            dl_hi[c, o:o + len(hi_src)] = hi_dl

    return dinv_pad, blk_lo, blk_hi, off_lo, off_hi, idx_lo, idx_hi, dl_lo, dl_hi


def _build_and_run(inputs_np, dinv_pad, blk_lo, blk_hi, off_lo, off_hi,
                   idx_lo, idx_hi, dl_lo, dl_hi, trace=False, sim=False):
    import concourse.bacc as bacc
    import concourse.mybir as mybir
    from concourse.tile import TileContext
    from concourse import bass, bass_utils, library_config
    from concourse.masks import make_identity

    x = inputs_np["x"]
    Ws = [np.asarray(inputs_np[k], np.float32) for k in ("W1", "W2", "W3")]
    bs = [np.asarray(inputs_np[k], np.float32) for k in ("b1", "b2", "b3")]
    # pad W3/b3 to 128 output features
    W3p = np.zeros((F, F), np.float32); W3p[:, :F_OUT] = Ws[2]
    b3p = np.zeros(F, np.float32); b3p[:F_OUT] = bs[2]
    Ws[2], bs[2] = W3p, b3p

    n_lo, n_hi = idx_lo.shape[1], idx_hi.shape[1]
    # gather groups of GROUP_WINDOWS windows
    groups = [list(range(g, min(g + GROUP_WINDOWS, N_WIN)))
              for g in range(0, N_WIN, GROUP_WINDOWS)]
    glo = [(int(off_lo[g[0]]), int(off_lo[g[-1] + 1])) for g in groups]
    ghi = [(int(off_hi[g[0]]), int(off_hi[g[-1] + 1])) for g in groups]
    cap_lo = max(b - a for a, b in glo) // 128
    cap_hi = max(1, max(b - a for a, b in ghi) // 128)

    nc = bacc.Bacc("TRN2", target_bir_lowering=False, debug=False, num_devices=N_CORES, num_swdge_queues=2)
    dt = mybir.dt

    # ---- kernel I/O -----------------------------------------------------
    t_xT = nc.dram_tensor("xT_own", [128, PER_CORE], dt.float32, kind="ExternalInput")
    t_W = [nc.dram_tensor(f"W{i+1}m", [F, F], dt.float32, kind="ExternalInput") for i in range(3)]
    t_b = [nc.dram_tensor(f"b{i+1}m", [128, F], dt.float32, kind="ExternalInput") for i in range(3)]
    t_dinv = nc.dram_tensor("dinv_own", [128, N_WIN], dt.float32, kind="ExternalInput")
    t_iota = nc.dram_tensor("iota", [128, 128], dt.bfloat16, kind="ExternalInput")
    t_ilo = nc.dram_tensor("idx_lo", [128, n_lo // 16], dt.int16, kind="ExternalInput")
    t_ihi = nc.dram_tensor("idx_hi", [128, max(1, n_hi // 16)], dt.int16, kind="ExternalInput")
    t_dlo = nc.dram_tensor("dl_lo", [128, n_lo // 128], dt.float32, kind="ExternalInput")
    t_dhi = nc.dram_tensor("dl_hi", [128, max(1, n_hi // 128)], dt.float32, kind="ExternalInput")
    t_out = nc.dram_tensor("h_out", [PER_CORE, F_OUT], dt.float32, kind="ExternalOutput")

    with TileContext(nc) as tc:
        nc.gpsimd.load_library(library_config.mlp)
        with tc.tile_pool(name="const", bufs=1) as cpool, \
             tc.tile_pool(name="state", bufs=1) as spool, \
             tc.tile_pool(name="gath", bufs=2) as gpool, \
             tc.tile_pool(name="work", bufs=3) as wpool, \
             tc.tile_pool(name="sbig", bufs=2) as sbig, \
             tc.tile_pool(name="psA", bufs=2, space="PSUM") as psA, \
             tc.tile_pool(name="psB", bufs=2, space="PSUM") as psB, \
             tc.tile_pool(name="psT", bufs=2, space="PSUM") as psT, \
             tc.tile_pool(name="dram", bufs=1, space="DRAM") as dpool:

            # ---- constants ----
            c_W = [cpool.tile([F, F], dt.float32, tag=f"W{i}", name=f"cW{i}") for i in range(3)]
            c_b = [cpool.tile([128, F], dt.float32, tag=f"b{i}", name=f"cb{i}") for i in range(3)]
            c_dinv = cpool.tile([128, N_WIN], dt.float32, tag="dinv", name="dinv")
            c_iota = cpool.tile([128, 128], dt.bfloat16, tag="iota", name="iota")
            c_ilo = cpool.tile([128, n_lo // 16], dt.int16, tag="ilo", name="ilo")
            c_ihi = cpool.tile([128, max(1, n_hi // 16)], dt.int16, tag="ihi", name="ihi")
            c_dlo = cpool.tile([128, n_lo // 128], dt.float32, tag="dlo", name="dlo")
            c_dhi = cpool.tile([128, max(1, n_hi // 128)], dt.float32, tag="dhi", name="dhi")
            c_ident = cpool.tile([128, 128], dt.float32, tag="ident", name="ident")
            for i in range(3):
                nc.sync.dma_start(c_W[i][:], t_W[i][:])
                nc.sync.dma_start(c_b[i][:], t_b[i][:])
            nc.sync.dma_start(c_dinv[:], t_dinv[:])
            nc.sync.dma_start(c_iota[:], t_iota[:])
            nc.sync.dma_start(c_ilo[:], t_ilo[:])
            nc.sync.dma_start(c_ihi[:], t_ihi[:])
            nc.sync.dma_start(c_dlo[:], t_dlo[:])
            nc.sync.dma_start(c_dhi[:], t_dhi[:])
            make_identity(nc, c_ident[:])

            # ---- persistent state ----
            hT = [spool.tile([128, PER_CORE], dt.float32, tag="hT_a", name="hT_a"),
                  spool.tile([128, PER_CORE], dt.float32, tag="hT_b", name="hT_b")]
            nc.sync.dma_start(hT[0][:], t_xT[:])
            y_sb = spool.tile([128, N_WIN, F], dt.bfloat16, tag="y_sb", name="y_sb")
            out_sb = spool.tile([128, N_WIN, F_OUT], dt.float32, tag="out_sb", name="out_sb")

            y_fulls = [dpool.tile([N_PAD, F], dt.bfloat16, addr_space="Shared",
                                  name=f"y_full{i}") for i in range(3)]
            ag_ins = [dpool.tile([PER_CORE, F], dt.bfloat16, name=f"ag_in{i}")
                      for i in range(3)]

            for layer in range(3):
                h_in = hT[layer % 2]
                h_out = hT[(layer + 1) % 2]
                # ---- phase A: y = dinv * (h @ W)  (own nodes) ----
                for t in range(N_WIN):
                    ps = psA.tile([128, F], dt.float32, tag="psA", space="PSUM")
                    nc.tensor.matmul(ps[:], lhsT=h_in[:, t * 128:(t + 1) * 128],
                                     rhs=c_W[layer][:], start=True, stop=True)
                    nc.vector.tensor_scalar(
                        out=y_sb[:, t, :], in0=ps[:],
                        scalar1=c_dinv[:, t:t + 1], scalar2=None,
                        op0=mybir.AluOpType.mult)
                ag_in = ag_ins[layer]
                y_full = y_fulls[layer]
                nc.sync.dma_start(
                    ag_in[:].rearrange("(t p) f -> p t f", p=128), y_sb[:])
                # ---- exchange: full y table ----
                nc.gpsimd.collective_compute(
                    "AllGather", mybir.AluOpType.bypass,
                    replica_groups=[list(range(N_CORES))],
                    ins=[ag_in.opt()], outs=[y_full.opt()])

                # ---- phase B: gather + one-hot matmul aggregation ----
                for gi, g in enumerate(groups):
                    lo_a, lo_b = glo[gi]
                    hi_a, hi_b = ghi[gi]
                    nlo = lo_b - lo_a
                    nhi = hi_b - hi_a
                    m_lo = gpool.tile([128, cap_lo, F], dt.bfloat16, tag="mlo", name="mlo")
                    m_hi = gpool.tile([128, cap_hi, F], dt.bfloat16, tag="mhi", name="mhi")
                    nc.gpsimd.dma_gather(
                        out_ap=m_lo[:, :nlo // 128, :], in_ap=y_full[:],
                        idxs_ap=c_ilo[:, lo_a // 16:lo_b // 16],
                        num_idxs=nlo, num_idxs_reg=nlo, elem_size=F,
                        queue_num=0, single_packet=False)
                    if nhi > 0:
                        nc.gpsimd.dma_gather(
                            out_ap=m_hi[:, :nhi // 128, :], in_ap=y_full[HI_BASE:, :],
                            idxs_ap=c_ihi[:, hi_a // 16:hi_b // 16],
                            num_idxs=nhi, num_idxs_reg=nhi, elem_size=F,
                            queue_num=1, single_packet=False)
                    for w in g:
                        nblk = int(blk_lo[w] + blk_hi[w])
                        agg = psB.tile([128, F], dt.float32, tag="agg", space="PSUM")
                        k = 0
                        for b in range(int(blk_lo[w])):
                            B = int(off_lo[w]) // 128 + b
                            S = wpool.tile([128, 128], dt.bfloat16, tag="S", name="S")
                            nc.vector.tensor_scalar(
                                out=S[:], in0=c_iota[:],
                                scalar1=c_dlo[:, B:B + 1], scalar2=None,
                                op0=mybir.AluOpType.is_equal)
                            nc.tensor.matmul(
                                agg[:], lhsT=S[:],
                                rhs=m_lo[:, B - lo_a // 128, :],
                                start=(k == 0), stop=(k == nblk - 1))
                            k += 1
                        for b in range(int(blk_hi[w])):
                            B = int(off_hi[w]) // 128 + b
                            S = wpool.tile([128, 128], dt.bfloat16, tag="S", name="S")
                            nc.vector.tensor_scalar(
                                out=S[:], in0=c_iota[:],
                                scalar1=c_dhi[:, B:B + 1], scalar2=None,
                                op0=mybir.AluOpType.is_equal)
                            nc.tensor.matmul(
                                agg[:], lhsT=S[:],
                                rhs=m_hi[:, B - hi_a // 128, :],
                                start=(k == 0), stop=(k == nblk - 1))
                            k += 1
                        # ---- epilogue: h = dinv*agg + b ----
                        hs = wpool.tile([128, F], dt.float32, tag="hs", name="hs")
                        nc.vector.tensor_scalar(
                            out=hs[:], in0=agg[:],
                            scalar1=c_dinv[:, w:w + 1], scalar2=None,
                            op0=mybir.AluOpType.mult)
                        if layer < 2:
                            hb = wpool.tile([128, F], dt.float32, tag="hb", name="hb")
                            nc.vector.tensor_add(hb[:], hs[:], c_b[layer][:])
                            tp = psT.tile([128, 128], dt.float32, tag="tp", space="PSUM")
                            nc.tensor.transpose(tp[:], hb[:], c_ident[:])
                            nc.vector.tensor_copy(
                                out=h_out[:, w * 128:(w + 1) * 128], in_=tp[:])
                        else:
                            hb = wpool.tile([128, F], dt.float32, tag="hb", name="hb")
                            nc.vector.tensor_add(hb[:], hs[:], c_b[layer][:])
                            nc.vector.tensor_scalar(
                                out=out_sb[:, w, :], in0=hb[:, :F_OUT],
                                scalar1=0.0, scalar2=None,
                                op0=mybir.AluOpType.max)
            nc.sync.dma_start(
                t_out[:].rearrange("(t p) f -> p t f", p=128), out_sb[:])

    nc.compile()

    # ---- per-core inputs ----
    xT_all = np.zeros((128, N_PAD), np.float32)
    xT_all[:, :N_NODES] = np.asarray(x, np.float32).T
    iota_m = np.broadcast_to(np.arange(128, dtype=np.float32), (128, 128)).astype(BF16)
    in_maps = []
    for c in range(N_CORES):
        rows = slice(c * PER_CORE, (c + 1) * PER_CORE)
        din = dinv_pad[rows].reshape(N_WIN, 128).T.copy()  # [128, N_WIN]
        in_map = {
            "xT_own": np.ascontiguousarray(xT_all[:, rows]),
            "dinv_own": din,
            "iota": iota_m.copy(),
            "idx_lo": _wrap_idx16(idx_lo[c]),
            "idx_hi": _wrap_idx16(idx_hi[c]) if n_hi else np.zeros((128, 1), np.int16),
            "dl_lo": dl_lo[c].reshape(-1, 128).T.copy(),
            "dl_hi": (dl_hi[c].reshape(-1, 128).T.copy() if n_hi
                      else np.zeros((128, 1), np.float32)),
        }
        for i in range(3):
            in_map[f"W{i+1}m"] = Ws[i].copy()
            in_map[f"b{i+1}m"] = np.broadcast_to(bs[i], (128, F)).copy()
        in_maps.append(in_map)

    if sim:
        from concourse.bass_interp import MultiCoreSim
        mcs = MultiCoreSim(nc, num_cores=N_CORES, trace=False,
                           require_finite=False, require_nnan=False)
        for ci, core in enumerate(mcs.cores.values()):
            for k, v in in_maps[ci].items():
                core.tensor(k)[:] = v
        mcs.simulate(check_with_hw=False)
        outs = [np.asarray(core.tensor("h_out"))
                for core in mcs.cores.values()]
        res = None
    else:
        res = bass_utils.run_bass_kernel_spmd(
            nc, in_maps, core_ids=list(range(N_CORES)), trace=trace)
        outs = [r["h_out"] for r in res.results]
    full = np.concatenate(outs, axis=0)[:N_NODES]
    return full, res


def kernel(**inputs) -> np.ndarray:
    edge_index = np.asarray(inputs["edge_index"])
    prep = _preprocess(edge_index)
    out, _ = _build_and_run(inputs, *prep)
    return out



# revision 2
# speedup vs baseline: 1.8406x; 1.8406x over previous
"""GCN 3-layer kernel for Trainium2, 8-core SPMD.

Math (per layer, PyG GCN convention with self-loops, factorized):
    deg[d]  = indegree(d) + 1;  dinv = deg^-1/2
    y       = dinv[:,None] * (h @ W)                    (per-node scale)
    agg[d]  = sum_{e: dst[e]=d} y[src[e]]  + y[d]       (self-loop direct)
    h_next  = dinv[:,None] * agg + b                    (+ relu on last layer)

Distribution: destination-sharded across 8 cores (6272 = 49*128 node slots
per core, padded to 50176 total).  Each core computes y for its own nodes,
an AllGather replicates the full y table (bf16) to every core's DRAM, then
each core gathers message rows with dma_gather (4 SWDGE queues) and
scatter-adds them with one-hot matmuls on the PE (PSUM accumulation per
128-dst window).

Key layout choices:
  * h state is kept feature-major [128 F, nodes].  Layers 1-2 aggregate in
    "aggT" orientation -- matmul(out=aggT[F, dst], lhsT=m[msg, F],
    rhs=S[msg, dst]) -- so the result lands feature-major with no PE
    transpose.  Layer 3 aggregates dst-major so the [N, 64] output can be
    DMA'd directly.
  * The one-hot S matrices for a whole window are built with a single wide
    tensor_tensor is_equal using stride-0 broadcast APs (iota bcast over
    blocks, dst-local bcast over the 128 compare lanes).
  * Self-loop contribution enters the PSUM chain as one identity matmul of
    the core's own y window (no gathered self-edges).

dma_gather indices are int16, so the y table is addressed via two base
offsets (row 0 for src < 25088, row 17408 for src >= 25088; 50176-17408 =
32768 rows exactly covers the int16 range).
"""

import numpy as np
import ml_dtypes

N_NODES = 50000
N_CORES = 8
PER_CORE = 6272            # 49 * 128
N_PAD = PER_CORE * N_CORES # 50176
N_WIN = PER_CORE // 128    # 49
HI_BASE = 17408            # hi gather base row; 50176-17408 = 32768
LO_HI_SPLIT = 25088        # src < split -> lo stream, else hi
F = 128                    # feature width (layer3 padded 64->128)
F_OUT = 64
GROUP_WINDOWS = 4          # windows per gather chunk

BF16 = ml_dtypes.bfloat16


def _wrap_idx16(idx: np.ndarray) -> np.ndarray:
    """Wrap a flat int16 index stream into the [128, n/16] layout dma_gather
    expects (element i at [i%16, i//16], replicated across the 8 groups of
    16 partitions)."""
    n = len(idx)
    assert n % 128 == 0
    cols = n // 16
    out = np.empty((128, cols), np.int16)
    w = idx.reshape(cols, 16).T  # [16, cols]
    for g in range(8):
        out[g * 16:(g + 1) * 16, :] = w
    return out


def _preprocess(edge_index: np.ndarray):
    """Host-side graph prep: degree norm, dst-sharding, per-window edge
    streams (lo/hi by source row), block padding shared across cores."""
    src = edge_index[0].astype(np.int64)
    dst = edge_index[1].astype(np.int64)
    deg = np.bincount(dst, minlength=N_NODES).astype(np.float64) + 1.0
    dinv = (1.0 / np.sqrt(deg)).astype(np.float32)
    dinv_pad = np.ones(N_PAD, np.float32)
    dinv_pad[:N_NODES] = dinv

    core_of = dst // PER_CORE
    win_of = (dst % PER_CORE) // 128
    dloc_of = dst % 128
    is_lo = src < LO_HI_SPLIT

    # sort once by (core, window)
    order = np.lexsort((dst, win_of, core_of))
    src_s, core_s, win_s, dloc_s, lo_s = (
        src[order], core_of[order], win_of[order], dloc_of[order], is_lo[order])

    # per (core, window, stream) counts
    counts = np.zeros((N_CORES, N_WIN, 2), np.int64)
    np.add.at(counts, (core_s, win_s, (~lo_s).astype(np.int64)), 1)
    # shared block counts per window (max over cores)
    blk_lo = -(-counts[:, :, 0].max(axis=0) // 128)  # [N_WIN]
    blk_hi = -(-counts[:, :, 1].max(axis=0) // 128)  # [N_WIN]
    nblk = blk_lo + blk_hi

    # slot offsets within each stream
    off_lo = np.concatenate([[0], np.cumsum(blk_lo * 128)])
    off_hi = np.concatenate([[0], np.cumsum(blk_hi * 128)])
    gboff = np.concatenate([[0], np.cumsum(nblk)])  # global block offset/window
    n_lo, n_hi = int(off_lo[-1]), int(off_hi[-1])
    tot_blk = int(gboff[-1])

    # fill per-core padded streams; merged dl stream ordered
    # (window, lo blocks, hi blocks)
    idx_lo = np.zeros((N_CORES, n_lo), np.int16)
    idx_hi = np.zeros((N_CORES, max(1, n_hi)), np.int16)
    dl_all = np.full((N_CORES, tot_blk * 128), 999.0, np.float32)

    # boundaries of (core, window) groups in the sorted arrays
    keys = core_s * N_WIN + win_s
    bounds = np.searchsorted(keys, np.arange(N_CORES * N_WIN + 1))
    for c in range(N_CORES):
        for w in range(N_WIN):
            k = c * N_WIN + w
            sl = slice(bounds[k], bounds[k + 1])
            s_src = src_s[sl]; s_dl = dloc_s[sl]; s_lo = lo_s[sl]
            lo_src = s_src[s_lo]; lo_dl = s_dl[s_lo]
            hi_src = s_src[~s_lo]; hi_dl = s_dl[~s_lo]
            o = off_lo[w]
            idx_lo[c, o:o + len(lo_src)] = lo_src.astype(np.int16)
            o = off_hi[w]
            idx_hi[c, o:o + len(hi_src)] = (hi_src - HI_BASE).astype(np.int16)
            gb = gboff[w] * 128
            dl_all[c, gb:gb + len(lo_dl)] = lo_dl
            gb2 = gboff[w] * 128 + int(blk_lo[w]) * 128
            dl_all[c, gb2:gb2 + len(hi_dl)] = hi_dl

    return dinv_pad, blk_lo, blk_hi, off_lo, off_hi, gboff, idx_lo, idx_hi, dl_all


def _build_and_run(inputs_np, dinv_pad, blk_lo, blk_hi, off_lo, off_hi,
                   gboff, idx_lo, idx_hi, dl_all, trace=False, sim=False):
    import concourse.bacc as bacc
    import concourse.mybir as mybir
    from concourse.tile import TileContext
    from concourse import bass, bass_utils, library_config

    x = inputs_np["x"]
    Ws = [np.asarray(inputs_np[k], np.float32) for k in ("W1", "W2", "W3")]
    bs = [np.asarray(inputs_np[k], np.float32) for k in ("b1", "b2", "b3")]
    # pad W3/b3 to 128 output features
    W3p = np.zeros((F, F), np.float32); W3p[:, :F_OUT] = Ws[2]
    b3p = np.zeros(F, np.float32); b3p[:F_OUT] = bs[2]
    Ws[2], bs[2] = W3p, b3p
    b_nonzero = [bool(np.any(b)) for b in bs]

    n_lo, n_hi = idx_lo.shape[1], idx_hi.shape[1] if np.any(blk_hi) else 0
    tot_blk = int(gboff[-1])
    maxblk = int((blk_lo + blk_hi).max())
    # gather groups of GROUP_WINDOWS windows
    groups = [list(range(g, min(g + GROUP_WINDOWS, N_WIN)))
              for g in range(0, N_WIN, GROUP_WINDOWS)]
    glo = [(int(off_lo[g[0]]), int(off_lo[g[-1] + 1])) for g in groups]
    ghi = [(int(off_hi[g[0]]), int(off_hi[g[-1] + 1])) for g in groups]
    cap_lo = max(b - a for a, b in glo) // 128
    cap_hi = max(1, max(b - a for a, b in ghi) // 128)

    nc = bacc.Bacc("TRN2", target_bir_lowering=False, debug=False,
                   num_devices=N_CORES, num_swdge_queues=4)
    dt = mybir.dt

    # ---- kernel I/O -----------------------------------------------------
    t_xT = nc.dram_tensor("xT_own", [128, PER_CORE], dt.bfloat16, kind="ExternalInput")
    t_W = [nc.dram_tensor(f"W{i+1}m", [F, F], dt.bfloat16, kind="ExternalInput") for i in range(3)]
    t_bc = [nc.dram_tensor(f"b{i+1}c", [128, 1], dt.float32, kind="ExternalInput") for i in range(3)]
    t_b3 = nc.dram_tensor("b3m", [128, F], dt.float32, kind="ExternalInput")
    t_dinv = nc.dram_tensor("dinv_own", [128, N_WIN], dt.float32, kind="ExternalInput")
    t_dinvbc = nc.dram_tensor("dinv_bc", [128, PER_CORE], dt.float32, kind="ExternalInput")
    t_iota = nc.dram_tensor("iota", [128, 128], dt.bfloat16, kind="ExternalInput")
    t_identb = nc.dram_tensor("identb", [128, 128], dt.bfloat16, kind="ExternalInput")
    t_ilo = nc.dram_tensor("idx_lo", [128, n_lo // 16], dt.int16, kind="ExternalInput")
    t_ihi = nc.dram_tensor("idx_hi", [128, max(1, n_hi // 16)], dt.int16, kind="ExternalInput")
    t_dl = nc.dram_tensor("dl_all", [128, tot_blk], dt.float32, kind="ExternalInput")
    t_out = nc.dram_tensor("h_out", [PER_CORE, F_OUT], dt.float32, kind="ExternalOutput")

    with TileContext(nc) as tc:
        nc.gpsimd.load_library(library_config.mlp)
        with tc.tile_pool(name="const", bufs=1) as cpool, \
             tc.tile_pool(name="state", bufs=1) as spool, \
             tc.tile_pool(name="gath", bufs=2) as gpool, \
             tc.tile_pool(name="spool", bufs=2) as spoolS, \
             tc.tile_pool(name="psA", bufs=2, space="PSUM") as psA, \
             tc.tile_pool(name="psB", bufs=2, space="PSUM") as psB, \
             tc.tile_pool(name="dram", bufs=1, space="DRAM") as dpool:

            # ---- constants ----
            c_W = [cpool.tile([F, F], dt.bfloat16, tag=f"W{i}", name=f"cW{i}") for i in range(3)]
            c_bc = [cpool.tile([128, 1], dt.float32, tag=f"bc{i}", name=f"cbc{i}") for i in range(3)]
            c_b3 = cpool.tile([128, F], dt.float32, tag="b3", name="cb3")
            c_dinv = cpool.tile([128, N_WIN], dt.float32, tag="dinv", name="dinv")
            c_dinvbc = cpool.tile([128, N_WIN, 128], dt.float32, tag="dinvbc", name="dinvbc")
            c_iota = cpool.tile([128, 128], dt.bfloat16, tag="iota", name="iota")
            c_identb = cpool.tile([128, 128], dt.bfloat16, tag="identb", name="identb")
            c_ilo = cpool.tile([128, n_lo // 16], dt.int16, tag="ilo", name="ilo")
            c_ihi = cpool.tile([128, max(1, n_hi // 16)], dt.int16, tag="ihi", name="ihi")
            c_dl = cpool.tile([128, tot_blk], dt.float32, tag="dl", name="dl")
            for i in range(3):
                nc.sync.dma_start(c_W[i][:], t_W[i][:])
                nc.sync.dma_start(c_bc[i][:], t_bc[i][:])
            nc.sync.dma_start(c_b3[:], t_b3[:])
            nc.sync.dma_start(c_dinv[:], t_dinv[:])
            nc.sync.dma_start(
                c_dinvbc[:].rearrange("p t f -> p (t f)"), t_dinvbc[:])
            nc.sync.dma_start(c_iota[:], t_iota[:])
            nc.sync.dma_start(c_identb[:], t_identb[:])
            nc.sync.dma_start(c_ilo[:], t_ilo[:])
            nc.sync.dma_start(c_ihi[:], t_ihi[:])
            nc.sync.dma_start(c_dl[:], t_dl[:])

            # ---- persistent state ----
            hT = [spool.tile([128, PER_CORE], dt.bfloat16, tag="hT_a", name="hT_a"),
                  spool.tile([128, PER_CORE], dt.bfloat16, tag="hT_b", name="hT_b")]
            nc.sync.dma_start(hT[0][:], t_xT[:])
            y_sb = spool.tile([128, N_WIN, F], dt.bfloat16, tag="y_sb", name="y_sb")
            out_sb = spool.tile([128, N_WIN, F_OUT], dt.float32, tag="out_sb", name="out_sb")

            y_fulls = [dpool.tile([N_PAD, F], dt.bfloat16, addr_space="Shared",
                                  name=f"y_full{i}") for i in range(3)]
            ag_ins = [dpool.tile([PER_CORE, F], dt.bfloat16, name=f"ag_in{i}")
                      for i in range(3)]

            for layer in range(3):
                h_in = hT[layer % 2]
                h_out = hT[(layer + 1) % 2]
                # ---- phase A: y = dinv * (h @ W)  (own nodes) ----
                for t in range(N_WIN):
                    ps = psA.tile([128, F], dt.float32, tag="psA", space="PSUM")
                    nc.tensor.matmul(ps[:], lhsT=h_in[:, t * 128:(t + 1) * 128],
                                     rhs=c_W[layer][:], start=True, stop=True)
                    nc.vector.tensor_scalar(
                        out=y_sb[:, t, :], in0=ps[:],
                        scalar1=c_dinv[:, t:t + 1], scalar2=None,
                        op0=mybir.AluOpType.mult)
                ag_in = ag_ins[layer]
                y_full = y_fulls[layer]
                nc.sync.dma_start(
                    ag_in[:].rearrange("(t p) f -> p t f", p=128), y_sb[:])
                # ---- exchange: full y table ----
                nc.gpsimd.collective_compute(
                    "AllGather", mybir.AluOpType.bypass,
                    replica_groups=[list(range(N_CORES))],
                    ins=[ag_in.opt()], outs=[y_full.opt()])

                # ---- phase B: gather + one-hot matmul aggregation ----
                for gi, g in enumerate(groups):
                    lo_a, lo_b = glo[gi]
                    hi_a, hi_b = ghi[gi]
                    m_lo = gpool.tile([128, cap_lo, F], dt.bfloat16, tag="mlo", name="mlo")
                    m_hi = gpool.tile([128, cap_hi, F], dt.bfloat16, tag="mhi", name="mhi")
                    # split each stream across two SWDGE queues
                    for (a, b, tile, base, idxs, q0, q2) in (
                            (lo_a, lo_b, m_lo, 0, c_ilo, 0, 2),
                            (hi_a, hi_b, m_hi, HI_BASE, c_ihi, 1, 3)):
                        nb = (b - a) // 128
                        if nb == 0:
                            continue
                        mid = a + (nb - nb // 2) * 128
                        for (aa, bb, qq) in ((a, mid, q0), (mid, b, q2)):
                            nn = bb - aa
                            if nn == 0:
                                continue
                            nc.gpsimd.dma_gather(
                                out_ap=tile[:, (aa - a) // 128:(bb - a) // 128, :],
                                in_ap=y_full[base:, :] if base else y_full[:],
                                idxs_ap=idxs[:, aa // 16:bb // 16],
                                num_idxs=nn, num_idxs_reg=nn, elem_size=F,
                                queue_num=qq, single_packet=False)
                    for w in g:
                        blo = int(blk_lo[w]); bhi = int(blk_hi[w])
                        nblk = blo + bhi
                        gb = int(gboff[w])
                        # one-hot S for the whole window in one wide op
                        S = spoolS.tile([128, maxblk, 128], dt.bfloat16,
                                        tag="S", name="S")
                        nc.vector.tensor_tensor(
                            out=S[:, :nblk, :],
                            in0=c_iota[:].unsqueeze(1).broadcast_to([128, nblk, 128]),
                            in1=c_dl[:, gb:gb + nblk].unsqueeze(2)
                                .broadcast_to([128, nblk, 128]),
                            op=mybir.AluOpType.is_equal)

                        def m_ap(j):
                            if j < blo:
                                return m_lo[:, (int(off_lo[w]) - lo_a) // 128 + j, :]
                            return m_hi[:, (int(off_hi[w]) - hi_a) // 128 + (j - blo), :]

                        agg = psB.tile([128, F], dt.float32, tag="agg", space="PSUM")
                        if layer < 2:
                            # aggT[f, d]: self-loop y^T then messages
                            nc.tensor.matmul(agg[:], lhsT=y_sb[:, w, :],
                                             rhs=c_identb[:],
                                             start=True, stop=(nblk == 0))
                            for j in range(nblk):
                                nc.tensor.matmul(agg[:], lhsT=m_ap(j),
                                                 rhs=S[:, j, :],
                                                 start=False, stop=(j == nblk - 1))
                            # epilogue: h = dinv_col * aggT (+ b)
                            if b_nonzero[layer]:
                                hs = spoolS.tile([128, F], dt.float32, tag="hs", name="hs")
                                nc.vector.tensor_tensor(
                                    out=hs[:], in0=agg[:], in1=c_dinvbc[:, w, :],
                                    op=mybir.AluOpType.mult)
                                nc.vector.tensor_scalar(
                                    out=h_out[:, w * 128:(w + 1) * 128], in0=hs[:],
                                    scalar1=c_bc[layer][:], scalar2=None,
                                    op0=mybir.AluOpType.add)
                            else:
                                nc.vector.tensor_tensor(
                                    out=h_out[:, w * 128:(w + 1) * 128],
                                    in0=agg[:], in1=c_dinvbc[:, w, :],
                                    op=mybir.AluOpType.mult)
                        else:
                            # agg[d, f]: self-loop then messages (dst-major)
                            nc.tensor.matmul(agg[:], lhsT=c_identb[:],
                                             rhs=y_sb[:, w, :],
                                             start=True, stop=(nblk == 0))
                            for j in range(nblk):
                                nc.tensor.matmul(agg[:], lhsT=S[:, j, :],
                                                 rhs=m_ap(j),
                                                 start=False, stop=(j == nblk - 1))
                            if b_nonzero[2]:
                                hs = spoolS.tile([128, F_OUT], dt.float32, tag="hs3", name="hs3")
                                nc.vector.scalar_tensor_tensor(
                                    out=hs[:], in0=agg[:, :F_OUT],
                                    scalar=c_dinv[:, w:w + 1], in1=c_b3[:, :F_OUT],
                                    op0=mybir.AluOpType.mult,
                                    op1=mybir.AluOpType.add)
                                nc.vector.tensor_scalar(
                                    out=out_sb[:, w, :], in0=hs[:],
                                    scalar1=0.0, scalar2=None,
                                    op0=mybir.AluOpType.max)
                            else:
                                # fused: relu(dinv * agg)
                                nc.vector.tensor_scalar(
                                    out=out_sb[:, w, :], in0=agg[:, :F_OUT],
                                    scalar1=c_dinv[:, w:w + 1], scalar2=0.0,
                                    op0=mybir.AluOpType.mult,
                                    op1=mybir.AluOpType.max)
            nc.sync.dma_start(
                t_out[:].rearrange("(t p) f -> p t f", p=128), out_sb[:])

    nc.compile()

    # ---- per-core inputs ----
    xT_all = np.zeros((128, N_PAD), np.float32)
    xT_all[:, :N_NODES] = np.asarray(x, np.float32).T
    iota_m = np.broadcast_to(np.arange(128, dtype=np.float32), (128, 128)).astype(BF16)
    ident_m = np.eye(128, dtype=np.float32).astype(BF16)
    in_maps = []
    for c in range(N_CORES):
        rows = slice(c * PER_CORE, (c + 1) * PER_CORE)
        din = dinv_pad[rows].reshape(N_WIN, 128).T.copy()  # [128, N_WIN]
        dinbc = np.broadcast_to(
            dinv_pad[rows].reshape(1, PER_CORE), (128, PER_CORE)).copy()
        in_map = {
            "xT_own": np.ascontiguousarray(xT_all[:, rows]).astype(BF16),
            "dinv_own": din,
            "dinv_bc": dinbc,
            "iota": iota_m.copy(),
            "identb": ident_m.copy(),
            "idx_lo": _wrap_idx16(idx_lo[c]),
            "idx_hi": _wrap_idx16(idx_hi[c]) if n_hi else np.zeros((128, 1), np.int16),
            "dl_all": dl_all[c].reshape(-1, 128).T.copy(),
        }
        for i in range(3):
            in_map[f"W{i+1}m"] = Ws[i].astype(BF16)
            in_map[f"b{i+1}c"] = bs[i][:128].reshape(128, 1).astype(np.float32) \
                if i < 2 else bs[i].reshape(128, 1).astype(np.float32)
        in_map["b3m"] = np.broadcast_to(bs[2], (128, F)).astype(np.float32)
        in_maps.append(in_map)

    if sim:
        from concourse.bass_interp import MultiCoreSim
        mcs = MultiCoreSim(nc, num_cores=N_CORES, trace=False,
                           require_finite=False, require_nnan=False)
        for ci, core in enumerate(mcs.cores.values()):
            for k, v in in_maps[ci].items():
                core.tensor(k)[:] = v
        mcs.simulate(check_with_hw=False)
        outs = [np.asarray(core.tensor("h_out"))
                for core in mcs.cores.values()]
        res = None
    else:
        res = bass_utils.run_bass_kernel_spmd(
            nc, in_maps, core_ids=list(range(N_CORES)), trace=trace)
        outs = [r["h_out"] for r in res.results]
    full = np.concatenate(outs, axis=0)[:N_NODES]
    return full, res


def kernel(**inputs) -> np.ndarray:
    edge_index = np.asarray(inputs["edge_index"])
    prep = _preprocess(edge_index)
    out, _ = _build_and_run(inputs, *prep)
    return out
